# revision 12
# baseline (speedup 1.0000x reference)
"""Trainium2 Bass kernel: 2:4 activation-sparse Linear (topk_masking).

Computes: out = prune_2to4(x.reshape(-1, d_in)) @ weight.T, reshaped back.

Strategy (8 NeuronCores, data-parallel over B*S rows):
  - Host packs x into a de-interleaved layout xp[gt, g, i, r] where the
    4 members of each contiguous d_in group-of-4 live in separate free-dim
    blocks at the same (partition, free) coordinates.  The 2:4 top-2-|.|
    mask then needs only elementwise max/min/is_ge ops on the VectorE
    (decisions in fp32, matching the fp32 reference top_k).
  - The pruned activation blocks [128 g, CH rows] are the moving operand
    of the TensorE matmul (contraction over partitions = d_in), with
    weight tiles [128 g, 128 n] stationary (host-packed, bf16).
  - Each stationary weight tile is loaded ONCE per output tile and reused
    for both row-chunk matmuls, which accumulate into two alternating
    PSUM banks.
  - The pruned-activation buffer is double-buffered across reps so the
    prune phase of iteration k+1 overlaps the matmul phase of iteration k.
  - PSUM accumulates in fp32; output tiles are written bf16 (outT), host
    re-transposes and upcasts.  Total rel err vs fp32 reference ~2.9e-3.

Roofline evidence (2026-08-10 session) — this kernel is at the 8-core HW
floor for its dataflow; do not chase the CoreSim gap:
  - CoreSim steady-state slope = 436 us/rep = the bf16 PE roofline
    (1,048,576 moving columns @ 2.4 GHz), PE 100% busy, prune/DMA fully
    hidden.  HW slope measures ~534-546 us/rep (same kernel measured
    617 us in an earlier session - day-to-day machine variance is real).
  - The ~25% HW-vs-sim gap is NOT per-LDWEIGHTS, NOT per-weight-switch,
    and NOT per-MM fixed overhead: microbenches with 2048/1088/512 LDWs,
    1024/512 weight switches, and 2048xN=512 vs 4096xN=256 matmuls all
    land at 548-555 us/rep.  A 1-core run of the same stream takes 462.7
    us (2.27 Gcol/s) vs 548.5 us with all 8 cores active (1.91 Gcol/s):
    the gap is chip-level clock/power throttling when all 8 PEs stream.
  - fp8 is dead for this tolerance (2e-2): e4m3 single-operand
    quantization error alone measures 2.65e-2 (pure fp8 both sides:
    3.75e-2), and any 2-pass correction costs the same as bf16.
  - Moving free dim is capped at 512 by the PSUM bank (512 fp32 = 2 KB),
    stationary reuse is capped at 2 by SBUF (spx double-buffer = 128
    KB/part), and sharding cannot reduce per-core PE columns (FLOPs are
    conserved).  dedupe_ldweights() below removes the redundant
    per-matmul LDWEIGHTS the tile scheduler emits; measured perf-neutral
    (LDW pipelines through the PE reorder window), so it is disabled.
  - A hand-rolled raw-Block matmul stream with ~zero semaphore traffic
    (bench_raw.py) measures 525.7 us/rep at 8 cores; this full kernel
    measured 525.5 us/rep in the same hour — the Tile framework adds no
    measurable overhead at 8 cores.  batch_mm_updates() (deferring the
    per-matmul PE_x sem-incs to chain ends) passes CoreSim but HANGS the
    hardware (NRT_EXEC_UNIT_UNRECOVERABLE on the first exec): the
    per-matmul increments are load-bearing for HW sync.  Keep it off.
"""

import sys

for _p in ("/opt/trn_rl_repo",):
    if _p not in sys.path:
        sys.path.insert(0, _p)

import numpy as np
import ml_dtypes

import concourse.bass as bass  # noqa: F401  (registers engine builders)
import concourse.mybir as mybir
import concourse.tile as tile
from concourse import bacc
from concourse.bass_utils import run_bass_kernel_spmd

F32 = mybir.dt.float32
BF16 = mybir.dt.bfloat16
AOP = mybir.AluOpType
ACT = mybir.ActivationFunctionType

DEDUPE_LDW = True
BATCH_MM_UPDATES = False  # hangs HW (NRT_EXEC_UNIT_UNRECOVERABLE) — do not enable

B, S, D_IN, D_OUT = 2, 4096, 4096, 4096
NCORES = 8
R = (B * S) // NCORES  # 1024 rows per core
NCHUNK = 2
GT = D_IN // 512  # 8 g-tiles of 128 groups
NT = D_OUT // 128  # 32 n-tiles


def _ldw_key(inst):
    a = inst.ins[0]
    return (
        a.memref,
        a.offset,
        str(a.ap),
        str(a.dtype),
        str(inst.perf_mode),
        str(inst.is_transpose),
        str(inst.tile_position),
    )


def dedupe_ldweights(nc):
    """Drop InstLdweights that reload the exact weights already resident.

    The tile scheduler emits one LDWEIGHTS per matmul even when consecutive
    matmuls share the same stationary AP.  Each reload costs ~128 PE cycles
    on HW (unmodeled by CoreSim; --enable-ldw-opt=false in the compile
    args).  Safe because between the kept LDW and the dropped one only
    InstMatmult instructions execute on the PE queue, and the stationary
    SBUF buffer is not rewritten while matmuls that read it are in flight
    (tile-pool semaphores guarantee this).  Waits/updates of a dropped LDW
    are merged into the following instruction so the semaphore protocol is
    preserved.
    """
    import concourse.mybir as _mybir

    n_dropped = 0
    for block in nc.m.functions[0].blocks:
        insts = block.instructions
        keep = []
        last_key = None
        pending_waits = []
        pending_updates = []
        for inst in insts:
            tname = type(inst).__name__
            eng = getattr(inst, "engine", None)
            is_pe = eng == _mybir.EngineType.PE
            if tname == "InstLdweights":
                k = _ldw_key(inst)
                if k == last_key:
                    si = inst.sync_info
                    if si is not None:
                        pending_waits.extend(si.on_wait)
                        pending_updates.extend(si.on_update)
                    n_dropped += 1
                    continue
                last_key = k
            elif is_pe and tname != "InstMatmult":
                last_key = None
            if pending_waits or pending_updates:
                si = inst.sync_info
                if si is None:
                    inst.sync_info = _mybir.SyncInfo(
                        on_wait=list(pending_waits), on_update=list(pending_updates)
                    )
                else:
                    si.on_wait = list(si.on_wait) + pending_waits
                    si.on_update = list(si.on_update) + pending_updates
                pending_waits = []
                pending_updates = []
            keep.append(inst)
        assert not pending_waits and not pending_updates
        block.instructions = keep
    return n_dropped


def batch_mm_updates(nc):
    """Defer per-matmul semaphore increments to accumulation-chain ends.

    Tile emits `sem-inc PE_x by 1` on every matmul (4096/2 reps) as its
    progress counter.  A hand-rolled sem-free stream measured ~3% faster
    (525.7 vs 541 us/rep, 8-core), so batch the increments: strip
    on_update from stop=False matmuls and merge the summed count into
    the next PE instruction that is not a stop=False matmul (the
    stop=True chain end, an LDWEIGHTS, or whatever follows).  Waits are
    untouched; the counter stays monotone and every threshold is still
    reached, only later — and every flush point precedes the earliest
    dependent PE wait by at least one nt-group, so no deadlock.
    """
    import concourse.mybir as _mybir

    n_batched = 0
    for block in nc.m.functions[0].blocks:
        pending = {}  # sem id -> (SyncUpdate template, total inc)
        for inst in block.instructions:
            if getattr(inst, "engine", None) != _mybir.EngineType.PE:
                continue
            tname = type(inst).__name__
            si = inst.sync_info
            if tname == "InstMatmult" and not inst.stop_tensor_calc:
                if si is not None and si.on_update:
                    for u in si.on_update:
                        if u.update_mode != "sem-inc" or u.update_reg is not None:
                            # unexpected shape: leave this instruction alone
                            break
                    else:
                        for u in si.on_update:
                            tmpl, tot = pending.get(u.id, (u, 0))
                            pending[u.id] = (tmpl, tot + u.update_value)
                            n_batched += 1
                        si.on_update = []
                continue
            if pending:
                if si is None:
                    si = _mybir.SyncInfo(on_wait=[], on_update=[])
                    inst.sync_info = si
                merged = list(si.on_update)
                for sem_id, (tmpl, tot) in pending.items():
                    for u in merged:
                        if (
                            u.id == sem_id
                            and u.update_mode == "sem-inc"
                            and u.update_reg is None
                        ):
                            u.update_value += tot
                            break
                    else:
                        tmpl.update_value = tot
                        merged.append(tmpl)
                si.on_update = merged
                pending = {}
        assert not pending, "unflushed PE sem updates at block end"
    return n_batched


def build(R=R, NCHUNK=NCHUNK, GT=GT, NT=NT, reps=1):
    CH = R // NCHUNK
    nc = bacc.Bacc("TRN2", target_bir_lowering=False, debug=False)
    xp = nc.dram_tensor("xp", [GT, 128, 4, R], F32, kind="ExternalInput").ap()
    wq = nc.dram_tensor("wq", [NT, 128, 4, GT, 128], BF16, kind="ExternalInput").ap()
    outT = nc.dram_tensor("outT", [NT, 128, R], BF16, kind="ExternalOutput").ap()

    with tile.TileContext(nc) as tc:
        with (
            tc.tile_pool(name="xa", bufs=2) as xpool,
            tc.tile_pool(name="ab", bufs=2) as abpool,
            tc.tile_pool(name="tmp", bufs=1) as tpool,
            tc.tile_pool(name="spx", bufs=2) as spool,
            tc.tile_pool(name="wb", bufs=2) as wpool,
            tc.tile_pool(name="ob", bufs=6) as opool,
            tc.tile_pool(name="ps", bufs=8, space="PSUM") as ppool,
        ):
            for _rep in range(reps):
                spx = spool.tile([128, NCHUNK, GT * 4, CH], BF16, tag="spx")
                # ---- prune phase (VectorE/ScalarE) ----
                for c in range(NCHUNK):
                    for gt in range(GT):
                        xa = xpool.tile([128, 4, CH], F32, tag="xa")
                        nc.sync.dma_start(xa, xp[gt, :, :, c * CH : (c + 1) * CH])
                        ab = abpool.tile([128, 4, CH], F32, tag="ab")
                        nc.scalar.activation(ab, xa, ACT.Abs)
                        h1 = tpool.tile([128, CH], F32, tag="h1")
                        l1 = tpool.tile([128, CH], F32, tag="l1")
                        h2 = tpool.tile([128, CH], F32, tag="h2")
                        l2 = tpool.tile([128, CH], F32, tag="l2")
                        nc.vector.tensor_tensor(h1, ab[:, 0], ab[:, 1], AOP.max)
                        nc.vector.tensor_tensor(l1, ab[:, 0], ab[:, 1], AOP.min)
                        nc.vector.tensor_tensor(h2, ab[:, 2], ab[:, 3], AOP.max)
                        nc.vector.tensor_tensor(l2, ab[:, 2], ab[:, 3], AOP.min)
                        nc.vector.tensor_tensor(h1, h1, h2, AOP.min)
                        nc.vector.tensor_tensor(l1, l1, l2, AOP.max)
                        # t = 2nd-largest |.| of each group of 4
                        nc.vector.tensor_tensor(h1, h1, l1, AOP.max)
                        tb = h1[:, None, :].broadcast_to([128, 4, CH])
                        nc.vector.tensor_tensor(ab, ab, tb, AOP.is_ge)
                        nc.vector.tensor_tensor(
                            spx[:, c, gt * 4 : (gt + 1) * 4, :], xa, ab, AOP.mult
                        )
                # ---- matmul phase (TensorE, stationary-weight reuse) ----
                for nt in range(NT):
                    wb = wpool.tile([128, 4, GT, 128], BF16, tag="wb")
                    nc.sync.dma_start(wb, wq[nt])
                    pss = [
                        ppool.tile([128, CH], F32, tag=f"ps{c}", name=f"ps{c}", bufs=4)
                        for c in range(NCHUNK)
                    ]
                    for kt in range(32):
                        gt, i = kt % GT, kt // GT
                        lhsT = wb[:, i, gt, :]
                        for c in range(NCHUNK):
                            nc.tensor.matmul(
                                pss[c],
                                lhsT,
                                spx[:, c, gt * 4 + i, :],
                                start=(kt == 0),
                                stop=(kt == 31),
                            )
                    for c in range(NCHUNK):
                        ob = opool.tile([128, CH], BF16, tag="ob")
                        nc.scalar.copy(ob, pss[c])
                        nc.sync.dma_start(outT[nt, :, c * CH : (c + 1) * CH], ob)
    if DEDUPE_LDW:
        dedupe_ldweights(nc)
    if BATCH_MM_UPDATES:
        batch_mm_updates(nc)
    nc.compile()
    return nc


def pack_x(x):
    # x [B, S, D_IN] fp32 -> per-core xp [NCORES, GT, 128, 4, R]
    xf = np.asarray(x, dtype=np.float32).reshape(NCORES, R, GT, 128, 4)
    return np.ascontiguousarray(xf.transpose(0, 2, 3, 4, 1))


def pack_w(w):
    # w [D_OUT, D_IN] fp32 -> wq [NT, 128, 4, GT, 128] bf16, free order (i, gt, n)
    wb = np.asarray(w).astype(ml_dtypes.bfloat16)
    return np.ascontiguousarray(
        wb.reshape(NT, 128, GT, 128, 4).transpose(0, 3, 4, 2, 1)
    )


def unpack_out(outs):
    # outs [NCORES, NT, 128, R] bf16 -> [B, S, D_OUT] fp32
    return np.ascontiguousarray(
        np.stack(outs).astype(np.float32).transpose(0, 3, 1, 2)
    ).reshape(B, S, D_OUT)


_NC = None


def _get_nc():
    global _NC
    if _NC is None:
        _NC = build()
    return _NC


def kernel(x, weight):
    nc = _get_nc()
    xp = pack_x(x)
    wq = pack_w(weight)
    in_maps = [{"xp": xp[c], "wq": wq} for c in range(NCORES)]
    res = run_bass_kernel_spmd(nc, in_maps, core_ids=list(range(NCORES)))
    outs = [res.results[c]["outT"] for c in range(NCORES)]
    return unpack_out(outs)



# revision 13
# speedup vs baseline: 1.0252x; 1.0252x over previous
"""Trainium2 Bass kernel: 2:4 activation-sparse Linear (topk_masking).

Computes: out = prune_2to4(x.reshape(-1, d_in)) @ weight.T, reshaped back.

Strategy (8 NeuronCores, data-parallel over B*S rows):
  - Host packs x into a de-interleaved layout xp[gt, g, i, r] where the
    4 members of each contiguous d_in group-of-4 live in separate free-dim
    blocks at the same (partition, free) coordinates.  The 2:4 top-2-|.|
    mask then needs only elementwise max/min/is_ge ops on the VectorE
    (decisions in fp32, matching the fp32 reference top_k).
  - The pruned activation blocks [128 g, CH rows] are the moving operand
    of the TensorE matmul (contraction over partitions = d_in), with
    weight tiles [128 g, 128 n] stationary (host-packed, bf16).
  - Each stationary weight tile is loaded ONCE per output tile and reused
    for both row-chunk matmuls, which accumulate into two alternating
    PSUM banks.
  - The pruned-activation buffer is double-buffered across reps so the
    prune phase of iteration k+1 overlaps the matmul phase of iteration k.
  - PSUM accumulates in fp32; output tiles are written bf16 (outT), host
    re-transposes and upcasts.  Total rel err vs fp32 reference ~2.9e-3.

Roofline evidence (2026-08-10 session) — this kernel is at the 8-core HW
floor for its dataflow; do not chase the CoreSim gap:
  - CoreSim steady-state slope = 436 us/rep = the bf16 PE roofline
    (1,048,576 moving columns @ 2.4 GHz), PE 100% busy, prune/DMA fully
    hidden.  HW slope measures ~534-546 us/rep (same kernel measured
    617 us in an earlier session - day-to-day machine variance is real).
  - The ~25% HW-vs-sim gap is NOT per-LDWEIGHTS, NOT per-weight-switch,
    and NOT per-MM fixed overhead: microbenches with 2048/1088/512 LDWs,
    1024/512 weight switches, and 2048xN=512 vs 4096xN=256 matmuls all
    land at 548-555 us/rep.  A 1-core run of the same stream takes 462.7
    us (2.27 Gcol/s) vs 548.5 us with all 8 cores active (1.91 Gcol/s):
    the gap is chip-level clock/power throttling when all 8 PEs stream.
  - fp8 is dead for this tolerance (2e-2): e4m3 single-operand
    quantization error alone measures 2.65e-2 (pure fp8 both sides:
    3.75e-2), and any 2-pass correction costs the same as bf16.
  - Moving free dim is capped at 512 by the PSUM bank (512 fp32 = 2 KB),
    stationary reuse is capped at 2 by SBUF (spx double-buffer = 128
    KB/part), and sharding cannot reduce per-core PE columns (FLOPs are
    conserved).  dedupe_ldweights() below removes the ~1000 redundant
    per-matmul LDWEIGHTS the tile scheduler emits.  ENABLED: neutral when
    the machine is fast (LDW pipelines through the PE reorder window),
    but a drift-robust interleaved A/B in a throttled machine state
    measured 553.1 vs 661.4 us/rep — when the chip throttles, the 1.2 GHz
    NX dispatch domain slows and redundant LDWs become the bottleneck, so
    dedup is weakly dominant (~0% fast days, ~15% slow days).
  - A hand-rolled raw-Block matmul stream with ~zero semaphore traffic
    (bench_raw.py) measures 525.7 us/rep at 8 cores; this full kernel
    measured 525.5 us/rep in the same hour — the Tile framework adds no
    measurable overhead at 8 cores.  batch_mm_updates() (deferring the
    per-matmul PE_x sem-incs to chain ends) passes CoreSim but HANGS the
    hardware (NRT_EXEC_UNIT_UNRECOVERABLE on the first exec): the
    per-matmul increments are load-bearing for HW sync.  Keep it off.
"""

import sys

for _p in ("/opt/trn_rl_repo",):
    if _p not in sys.path:
        sys.path.insert(0, _p)

import numpy as np
import ml_dtypes

import concourse.bass as bass  # noqa: F401  (registers engine builders)
import concourse.mybir as mybir
import concourse.tile as tile
from concourse import bacc
from concourse.bass_utils import run_bass_kernel_spmd

F32 = mybir.dt.float32
BF16 = mybir.dt.bfloat16
AOP = mybir.AluOpType
ACT = mybir.ActivationFunctionType

DEDUPE_LDW = True
BATCH_MM_UPDATES = False  # hangs HW (NRT_EXEC_UNIT_UNRECOVERABLE) — do not enable

B, S, D_IN, D_OUT = 2, 4096, 4096, 4096
NCORES = 8
R = (B * S) // NCORES  # 1024 rows per core
NCHUNK = 2
GT = D_IN // 512  # 8 g-tiles of 128 groups
NT = D_OUT // 128  # 32 n-tiles


def _ldw_key(inst):
    a = inst.ins[0]
    return (
        a.memref,
        a.offset,
        str(a.ap),
        str(a.dtype),
        str(inst.perf_mode),
        str(inst.is_transpose),
        str(inst.tile_position),
    )


def dedupe_ldweights(nc):
    """Drop InstLdweights that reload the exact weights already resident.

    The tile scheduler emits one LDWEIGHTS per matmul even when consecutive
    matmuls share the same stationary AP.  Each reload costs ~128 PE cycles
    on HW (unmodeled by CoreSim; --enable-ldw-opt=false in the compile
    args).  Safe because between the kept LDW and the dropped one only
    InstMatmult instructions execute on the PE queue, and the stationary
    SBUF buffer is not rewritten while matmuls that read it are in flight
    (tile-pool semaphores guarantee this).  Waits/updates of a dropped LDW
    are merged into the following instruction so the semaphore protocol is
    preserved.
    """
    import concourse.mybir as _mybir

    n_dropped = 0
    for block in nc.m.functions[0].blocks:
        insts = block.instructions
        keep = []
        last_key = None
        pending_waits = []
        pending_updates = []
        for inst in insts:
            tname = type(inst).__name__
            eng = getattr(inst, "engine", None)
            is_pe = eng == _mybir.EngineType.PE
            if tname == "InstLdweights":
                k = _ldw_key(inst)
                if k == last_key:
                    si = inst.sync_info
                    if si is not None:
                        pending_waits.extend(si.on_wait)
                        pending_updates.extend(si.on_update)
                    n_dropped += 1
                    continue
                last_key = k
            elif is_pe and tname != "InstMatmult":
                last_key = None
            if pending_waits or pending_updates:
                si = inst.sync_info
                if si is None:
                    inst.sync_info = _mybir.SyncInfo(
                        on_wait=list(pending_waits), on_update=list(pending_updates)
                    )
                else:
                    si.on_wait = list(si.on_wait) + pending_waits
                    si.on_update = list(si.on_update) + pending_updates
                pending_waits = []
                pending_updates = []
            keep.append(inst)
        assert not pending_waits and not pending_updates
        block.instructions = keep
    return n_dropped


def batch_mm_updates(nc):
    """Defer per-matmul semaphore increments to accumulation-chain ends.

    Tile emits `sem-inc PE_x by 1` on every matmul (4096/2 reps) as its
    progress counter.  A hand-rolled sem-free stream measured ~3% faster
    (525.7 vs 541 us/rep, 8-core), so batch the increments: strip
    on_update from stop=False matmuls and merge the summed count into
    the next PE instruction that is not a stop=False matmul (the
    stop=True chain end, an LDWEIGHTS, or whatever follows).  Waits are
    untouched; the counter stays monotone and every threshold is still
    reached, only later — and every flush point precedes the earliest
    dependent PE wait by at least one nt-group, so no deadlock.
    """
    import concourse.mybir as _mybir

    n_batched = 0
    for block in nc.m.functions[0].blocks:
        pending = {}  # sem id -> (SyncUpdate template, total inc)
        for inst in block.instructions:
            if getattr(inst, "engine", None) != _mybir.EngineType.PE:
                continue
            tname = type(inst).__name__
            si = inst.sync_info
            if tname == "InstMatmult" and not inst.stop_tensor_calc:
                if si is not None and si.on_update:
                    for u in si.on_update:
                        if u.update_mode != "sem-inc" or u.update_reg is not None:
                            # unexpected shape: leave this instruction alone
                            break
                    else:
                        for u in si.on_update:
                            tmpl, tot = pending.get(u.id, (u, 0))
                            pending[u.id] = (tmpl, tot + u.update_value)
                            n_batched += 1
                        si.on_update = []
                continue
            if pending:
                if si is None:
                    si = _mybir.SyncInfo(on_wait=[], on_update=[])
                    inst.sync_info = si
                merged = list(si.on_update)
                for sem_id, (tmpl, tot) in pending.items():
                    for u in merged:
                        if (
                            u.id == sem_id
                            and u.update_mode == "sem-inc"
                            and u.update_reg is None
                        ):
                            u.update_value += tot
                            break
                    else:
                        tmpl.update_value = tot
                        merged.append(tmpl)
                si.on_update = merged
                pending = {}
        assert not pending, "unflushed PE sem updates at block end"
    return n_batched


def build(R=R, NCHUNK=NCHUNK, GT=GT, NT=NT, reps=1):
    CH = R // NCHUNK
    nc = bacc.Bacc("TRN2", target_bir_lowering=False, debug=False)
    xp = nc.dram_tensor("xp", [GT, 128, 4, R], F32, kind="ExternalInput").ap()
    wq = nc.dram_tensor("wq", [NT, 128, 4, GT, 128], BF16, kind="ExternalInput").ap()
    outT = nc.dram_tensor("outT", [NT, 128, R], BF16, kind="ExternalOutput").ap()

    with tile.TileContext(nc) as tc:
        with (
            tc.tile_pool(name="xa", bufs=2) as xpool,
            tc.tile_pool(name="ab", bufs=2) as abpool,
            tc.tile_pool(name="tmp", bufs=1) as tpool,
            tc.tile_pool(name="spx", bufs=2) as spool,
            tc.tile_pool(name="wb", bufs=2) as wpool,
            tc.tile_pool(name="ob", bufs=6) as opool,
            tc.tile_pool(name="ps", bufs=8, space="PSUM") as ppool,
        ):
            for _rep in range(reps):
                spx = spool.tile([128, NCHUNK, GT * 4, CH], BF16, tag="spx")
                # ---- prune phase (VectorE/ScalarE) ----
                for c in range(NCHUNK):
                    for gt in range(GT):
                        xa = xpool.tile([128, 4, CH], F32, tag="xa")
                        nc.sync.dma_start(xa, xp[gt, :, :, c * CH : (c + 1) * CH])
                        ab = abpool.tile([128, 4, CH], F32, tag="ab")
                        nc.scalar.activation(ab, xa, ACT.Abs)
                        h1 = tpool.tile([128, CH], F32, tag="h1")
                        l1 = tpool.tile([128, CH], F32, tag="l1")
                        h2 = tpool.tile([128, CH], F32, tag="h2")
                        l2 = tpool.tile([128, CH], F32, tag="l2")
                        nc.vector.tensor_tensor(h1, ab[:, 0], ab[:, 1], AOP.max)
                        nc.vector.tensor_tensor(l1, ab[:, 0], ab[:, 1], AOP.min)
                        nc.vector.tensor_tensor(h2, ab[:, 2], ab[:, 3], AOP.max)
                        nc.vector.tensor_tensor(l2, ab[:, 2], ab[:, 3], AOP.min)
                        nc.vector.tensor_tensor(h1, h1, h2, AOP.min)
                        nc.vector.tensor_tensor(l1, l1, l2, AOP.max)
                        # t = 2nd-largest |.| of each group of 4
                        nc.vector.tensor_tensor(h1, h1, l1, AOP.max)
                        tb = h1[:, None, :].broadcast_to([128, 4, CH])
                        nc.vector.tensor_tensor(ab, ab, tb, AOP.is_ge)
                        nc.vector.tensor_tensor(
                            spx[:, c, gt * 4 : (gt + 1) * 4, :], xa, ab, AOP.mult
                        )
                # ---- matmul phase (TensorE, stationary-weight reuse) ----
                for nt in range(NT):
                    wb = wpool.tile([128, 4, GT, 128], BF16, tag="wb")
                    nc.sync.dma_start(wb, wq[nt])
                    pss = [
                        ppool.tile([128, CH], F32, tag=f"ps{c}", name=f"ps{c}", bufs=4)
                        for c in range(NCHUNK)
                    ]
                    for kt in range(32):
                        gt, i = kt % GT, kt // GT
                        lhsT = wb[:, i, gt, :]
                        for c in range(NCHUNK):
                            nc.tensor.matmul(
                                pss[c],
                                lhsT,
                                spx[:, c, gt * 4 + i, :],
                                start=(kt == 0),
                                stop=(kt == 31),
                            )
                    for c in range(NCHUNK):
                        ob = opool.tile([128, CH], BF16, tag="ob")
                        nc.scalar.copy(ob, pss[c])
                        nc.sync.dma_start(outT[nt, :, c * CH : (c + 1) * CH], ob)
    if DEDUPE_LDW:
        dedupe_ldweights(nc)
    if BATCH_MM_UPDATES:
        batch_mm_updates(nc)
    nc.compile()
    return nc


def pack_x(x):
    # x [B, S, D_IN] fp32 -> per-core xp [NCORES, GT, 128, 4, R]
    xf = np.asarray(x, dtype=np.float32).reshape(NCORES, R, GT, 128, 4)
    return np.ascontiguousarray(xf.transpose(0, 2, 3, 4, 1))


def pack_w(w):
    # w [D_OUT, D_IN] fp32 -> wq [NT, 128, 4, GT, 128] bf16, free order (i, gt, n)
    wb = np.asarray(w).astype(ml_dtypes.bfloat16)
    return np.ascontiguousarray(
        wb.reshape(NT, 128, GT, 128, 4).transpose(0, 3, 4, 2, 1)
    )


def unpack_out(outs):
    # outs [NCORES, NT, 128, R] bf16 -> [B, S, D_OUT] fp32
    return np.ascontiguousarray(
        np.stack(outs).astype(np.float32).transpose(0, 3, 1, 2)
    ).reshape(B, S, D_OUT)


_NC = None


def _get_nc():
    global _NC
    if _NC is None:
        _NC = build()
    return _NC


def kernel(x, weight):
    nc = _get_nc()
    xp = pack_x(x)
    wq = pack_w(weight)
    in_maps = [{"xp": xp[c], "wq": wq} for c in range(NCORES)]
    res = run_bass_kernel_spmd(nc, in_maps, core_ids=list(range(NCORES)))
    outs = [res.results[c]["outT"] for c in range(NCORES)]
    return unpack_out(outs)

